# revision 14
# baseline (speedup 1.0000x reference)
"""MLAttention (label-pooling attention) Trainium2 Bass kernel.

Computes, for full inputs:
    scores = einsum('bsh,lh->bls', inputs, W)
    scores = where(mask==0, -inf, scores)
    attn   = softmax(scores, axis=-1)
    out    = einsum('bls,bsh->blh', attn, inputs)

Label-parallel across 8 NeuronCores: L=28415 padded to 28672 = 8*3584.
Each core gets its own W shard [3584, 512]; inputs/masks replicated.
Host concatenates the 8 per-core outputs [B, 3584, H] and trims to L.

Transpose-free dataflow. Scores are computed TRANSPOSED, in [s, l]
layout, so the exp() tile is directly the stationary operand of the
second matmul -- no PE transposes. The softmax mask is folded into the
exp bias (per-partition = per-s). Row-sums (softmax denominators) come
from N=1 matmuls against a ones column that reuse mm2's already-loaded
stationary, accumulating into a separate PSUM bank; normalization
happens in the final ACT copy via a per-partition reciprocal scale.

Matmul operands are host-cast: mm2 in bf16; mm1 either bf16 ("c") or
fp8e4m3 with DoubleRow perf mode ("d", 2 MACs/cell/cycle, halves mm1
stream time; W is pre-scaled by 2^14 into fp8 range and the exp
activation descales via its free affine scale). Accumulation is fp32
in PSUM; exp() runs on ACT in fp32 from PSUM and rounds to bf16.

Input DMAs are issued on the ACT HWDGE queue, W-shard and output DMAs
on the sync queue, so the two big fill-phase loads stream in parallel.
A one-step software pipeline (group g's mm1 emitted before group
g-1's mm2) keeps the in-order PE queue full while g's exp chain
completes on ACT.
"""

from contextlib import ExitStack

import ml_dtypes
import numpy as np

import concourse.bass as bass
import concourse.mybir as mybir
import concourse.tile as tile
from concourse import bacc, bass_utils
from concourse.bass import ds, ts

F32 = mybir.dt.float32
BF16 = mybir.dt.bfloat16
FP8 = mybir.dt.float8e4

# Problem shapes (hardcoded per contract).
B, S, H, L = 4, 512, 512, 28415
N_CORES = 8
LSH = 3584               # per-core padded label count (28 tiles of 128)
L_PAD = LSH * N_CORES    # 28672
W_SCALE = 2.0 ** 14      # fp8 variant: host premultiplies W, exp descales


def build_module(b_sz=B, s_sz=S, h_sz=H, lsh=LSH, n_devices=N_CORES,
                 mm1_fp8=False):
    P = 128
    KH = h_sz // P   # H contraction chunks (mm1)
    KS = s_sz // P   # S contraction chunks (mm2) == score s-tiles
    LG = 512         # label group per step
    NG = lsh // LG   # groups per batch
    NSUB = LG // P   # 128-label tiles per group
    mm1_dt = FP8 if mm1_fp8 else BF16

    nc = bacc.Bacc(
        "TRN2", target_bir_lowering=False, debug=False, num_devices=n_devices
    )
    x_d = nc.dram_tensor("x", [b_sz, s_sz, h_sz], BF16, kind="ExternalInput").ap()
    xt_d = nc.dram_tensor("xt", [b_sz, h_sz, s_sz], mm1_dt, kind="ExternalInput").ap()
    wt_d = nc.dram_tensor("wt", [h_sz, lsh], mm1_dt, kind="ExternalInput").ap()
    m_d = nc.dram_tensor("m", [b_sz, s_sz], BF16, kind="ExternalInput").ap()
    o_d = nc.dram_tensor("o", [b_sz, lsh, h_sz], F32, kind="ExternalOutput").ap()

    with tile.TileContext(nc) as tc, ExitStack() as ctx:
        const = ctx.enter_context(tc.tile_pool(name="const", bufs=1))
        res = ctx.enter_context(tc.tile_pool(name="res", bufs=1))
        work = ctx.enter_context(tc.tile_pool(name="work", bufs=3))
        psum = ctx.enter_context(tc.tile_pool(name="psum", bufs=2, space="PSUM"))

        # Resident SBUF tensors (narrow dtypes straight from DMA, no casts).
        # Masking is exact and multiplicative: host pre-masks x rows (so
        # masked s contribute 0 to mm2) and the rowsum tiny-matmuls use the
        # mask column instead of ones (so masked s leave the denominator).
        # exp() therefore needs no per-chunk bias and fuses into a single
        # ACTIVATE per group.
        XT = res.tile([P, b_sz, KH, s_sz], mm1_dt)  # XT[h%128, b, h//128, s]
        XB = res.tile([P, b_sz, KS, h_sz], BF16)    # XB[s%128, b, s//128, h] (masked)
        WT = res.tile([P, KH, lsh], mm1_dt)         # WT[h%128, h//128, l]
        MC = res.tile([P, b_sz, KS], BF16)          # mask column per s

        def mask_setup():
            nc.sync.dma_start(MC[:], m_d.rearrange("b (c p) -> p b c", p=P))

        # Big input loads go on the ACT HWDGE queue so they stream in
        # parallel with the W-shard loads on the sync queue.
        def xt_setup(b):
            nc.scalar.dma_start(
                XT[:, b], xt_d[b].rearrange("(k p) s -> p k s", p=P)
            )

        def xb_setup(b):
            nc.scalar.dma_start(
                XB[:, b], x_d[b].rearrange("(c p) h -> p c h", p=P)
            )

        def w_setup(g):
            nc.sync.dma_start(
                WT[:, :, ts(g, LG)],
                wt_d[:, ts(g, LG)].rearrange("(k p) l -> p k l", p=P),
            )

        exp_scale = (1.0 / W_SCALE) if mm1_fp8 else 1.0

        def front(b, g, chunked_exp=False):
            """mm1 (scoresT chunks) + exp for group (b, g).

            exp runs as one fused ACTIVATE over all 4 chunks (saves the
            3x352-cycle ACT ramp); the last group uses per-chunk exps so
            the tail's mm2 isn't serialized behind a 2us fused exp."""
            ps_sct = psum.tile([P, KS, LG], F32, tag="ps_sct", bufs=1)
            exp_g = work.tile([P, KS, LG], BF16, tag="exp", bufs=2)
            for st in range(KS):
                if mm1_fp8:
                    for k2 in range(0, KH, 2):
                        nc.tensor.matmul(
                            ps_sct[:, st, :],
                            XT[:, b, ds(k2, 2), ts(st, P)],
                            WT[:, ds(k2, 2), ts(g, LG)],
                            start=(k2 == 0),
                            stop=(k2 == KH - 2),
                            perf_mode=mybir.MatmulPerfMode.DoubleRow,
                        )
                else:
                    for k in range(KH):
                        nc.tensor.matmul(
                            ps_sct[:, st, :],
                            XT[:, b, k, ts(st, P)],
                            WT[:, k, ts(g, LG)],
                            start=(k == 0),
                            stop=(k == KH - 1),
                        )
                if chunked_exp:
                    nc.scalar.activation(
                        exp_g[:, st, :], ps_sct[:, st, :],
                        mybir.ActivationFunctionType.Exp,
                        scale=exp_scale,
                    )
            if not chunked_exp:
                nc.scalar.activation(
                    exp_g[:], ps_sct[:],
                    mybir.ActivationFunctionType.Exp,
                    scale=exp_scale,
                )
            return exp_g

        def back(b, g, exp_g):
            """mm2 + rowsums + normalize + store for group (b, g)."""
            ps_sums = psum.tile([P, NSUB], F32, tag="ps_sums", bufs=2)
            recips = work.tile([P, NSUB], F32, tag="recips", bufs=4)
            for lt in range(NSUB):
                ps_out = psum.tile([P, h_sz], F32, tag="ps_out", bufs=2)
                for sc in range(KS):
                    stat = exp_g[:, sc, ts(lt, P)]
                    nc.tensor.matmul(
                        ps_out[:], stat, XB[:, b, sc, :],
                        start=(sc == 0), stop=(sc == KS - 1),
                    )
                    nc.tensor.matmul(
                        ps_sums[:, lt : lt + 1], stat, MC[:, b, sc : sc + 1],
                        start=(sc == 0), stop=(sc == KS - 1),
                    )
                nc.vector.reciprocal(
                    recips[:, lt : lt + 1], ps_sums[:, lt : lt + 1]
                )
                out_t = work.tile([P, h_sz], F32, tag="out", bufs=3)
                nc.scalar.activation(
                    out_t[:], ps_out[:],
                    mybir.ActivationFunctionType.Copy,
                    scale=recips[:, lt : lt + 1],
                )
                nc.sync.dma_start(o_d[b, ds(g * LG + lt * P, P), :], out_t[:])

        # ---- emission. Fill: xt0 streams on the ACT queue while mask+w0+w1
        # go on sync; xb0 follows xt0 on the ACT queue (first read is one
        # step later, in back(0,0)). W groups stream two ahead during b=0;
        # b+1 inputs prefetch early in batch b's pass.
        mask_setup()
        w_setup(0)
        xt_setup(0)
        xb_setup(0)
        w_setup(1)

        pend = [None]
        for b in range(b_sz):
            for g in range(NG):
                if b == 0 and g + 2 < NG:
                    w_setup(g + 2)
                last = (b == b_sz - 1) and (g == NG - 1)
                exp_g = front(b, g, chunked_exp=last)
                if b < b_sz - 1 and g == 0:
                    xt_setup(b + 1)
                    xb_setup(b + 1)
                if pend[0] is not None:
                    back(*pend[0])
                pend[0] = (b, g, exp_g)
        back(*pend[0])

    nc.compile()
    return nc


_CACHE = {}

VARIANT = "d"  # "c": bf16 mm1; "d": fp8 DoubleRow mm1


def _get_module():
    if VARIANT not in _CACHE:
        _CACHE[VARIANT] = build_module(mm1_fp8=(VARIANT == "d"))
    return _CACHE[VARIANT]


def _run(inputs: np.ndarray, masks: np.ndarray, W: np.ndarray, **spmd_kwargs):
    """Run on 8 cores; returns (full output, BassKernelResults)."""
    nc = _get_module()

    x32 = np.ascontiguousarray(inputs, dtype=np.float32)
    xt32 = np.ascontiguousarray(np.swapaxes(x32, 1, 2))
    mf = np.ascontiguousarray(masks, dtype=np.float32)
    # Pre-mask x rows: masked s contribute 0 to mm2 and (via the mask
    # column in the rowsum matmuls) to the softmax denominator -- exact
    # equivalent of -inf score masking.
    x = (x32 * mf[:, :, None]).astype(ml_dtypes.bfloat16)
    mc = mf.astype(ml_dtypes.bfloat16)
    wt_pad = np.zeros((H, L_PAD), dtype=np.float32)
    wt_pad[:, :L] = W.T
    if VARIANT == "d":
        xt = xt32.astype(ml_dtypes.float8_e4m3)
        wt_n = np.clip(wt_pad * W_SCALE, -240.0, 240.0).astype(
            ml_dtypes.float8_e4m3
        )
    else:
        xt = xt32.astype(ml_dtypes.bfloat16)
        wt_n = wt_pad.astype(ml_dtypes.bfloat16)

    in_maps = [
        {
            "x": x,
            "xt": xt,
            "m": mc,
            "wt": np.ascontiguousarray(wt_n[:, c * LSH : (c + 1) * LSH]),
        }
        for c in range(N_CORES)
    ]
    res = bass_utils.run_bass_kernel_spmd(
        nc, in_maps, core_ids=list(range(N_CORES)), **spmd_kwargs
    )
    out = np.concatenate([res.results[c]["o"] for c in range(N_CORES)], axis=1)
    return np.ascontiguousarray(out[:, :L, :]), res


def kernel(inputs: np.ndarray, masks: np.ndarray, W: np.ndarray) -> np.ndarray:
    out, _ = _run(inputs, masks, W)
    return out


# revision 21
# speedup vs baseline: 1.2163x; 1.2163x over previous
"""MLAttention (label-pooling attention) Trainium2 Bass kernel.

Computes, for full inputs:
    scores = einsum('bsh,lh->bls', inputs, W)
    scores = where(mask==0, -inf, scores)
    attn   = softmax(scores, axis=-1)
    out    = einsum('bls,bsh->blh', attn, inputs)

Label-parallel across 8 NeuronCores: L=28415 padded to 28672 = 8*3584.
Each core gets its own W shard [3584, 512]; inputs/masks replicated.
Host concatenates the 8 per-core outputs [B, 3584, H] and trims to L.

Transpose-free dataflow. Scores are computed TRANSPOSED, in [s, l]
layout, so the exp() tile is directly the stationary operand of the
second matmul -- no PE transposes. The softmax mask is folded into the
exp bias (per-partition = per-s). Row-sums (softmax denominators) come
from N=1 matmuls against a ones column that reuse mm2's already-loaded
stationary, accumulating into a separate PSUM bank; normalization
happens in the final ACT copy via a per-partition reciprocal scale.

Matmul operands are host-cast: mm2 in bf16; mm1 either bf16 ("c") or
fp8e4m3 with DoubleRow perf mode ("d", 2 MACs/cell/cycle, halves mm1
stream time; W is pre-scaled by 2^14 into fp8 range and the exp
activation descales via its free affine scale). Accumulation is fp32
in PSUM; exp() runs on ACT in fp32 from PSUM and rounds to bf16.

Input DMAs are issued on the ACT HWDGE queue, W-shard and output DMAs
on the sync queue, so the two big fill-phase loads stream in parallel.
A one-step software pipeline (group g's mm1 emitted before group
g-1's mm2) keeps the in-order PE queue full while g's exp chain
completes on ACT.
"""

from contextlib import ExitStack

import ml_dtypes
import numpy as np

import concourse.bass as bass
import concourse.mybir as mybir
import concourse.tile as tile
from concourse import bacc, bass_utils
from concourse.bass import ds, ts

F32 = mybir.dt.float32
BF16 = mybir.dt.bfloat16
FP8 = mybir.dt.float8e4

# Problem shapes (hardcoded per contract).
B, S, H, L = 4, 512, 512, 28415
N_CORES = 8
LSH = 3584               # per-core padded label count (28 tiles of 128)
L_PAD = LSH * N_CORES    # 28672
W_SCALE = 2.0 ** 14      # fp8 variant: host premultiplies W, exp descales


def build_module(b_sz=B, s_sz=S, h_sz=H, lsh=LSH, n_devices=N_CORES,
                 mm1_fp8=False):
    P = 128
    KH = h_sz // P   # H contraction chunks (mm1)
    KS = s_sz // P   # S contraction chunks (mm2) == score s-tiles
    LG = 512         # label group per step
    NG = lsh // LG   # groups per batch
    NSUB = LG // P   # 128-label tiles per group
    mm1_dt = FP8 if mm1_fp8 else BF16

    nc = bacc.Bacc(
        "TRN2", target_bir_lowering=False, debug=False, num_devices=n_devices
    )
    # Inputs are host-packed into the exact SBUF per-partition layouts so
    # every DMA moves 2-4 KiB contiguous runs per partition (full HBM BW).
    x_d = nc.dram_tensor(
        "x", [b_sz, P, KS, h_sz], BF16, kind="ExternalInput"
    ).ap()
    xt_d = nc.dram_tensor(
        "xt", [b_sz, P, KH, s_sz], mm1_dt, kind="ExternalInput"
    ).ap()
    wt_d = nc.dram_tensor(
        "wt", [NG, P, KH, LG], mm1_dt, kind="ExternalInput"
    ).ap()
    m_d = nc.dram_tensor("m", [P, b_sz, KS], BF16, kind="ExternalInput").ap()
    o_d = nc.dram_tensor("o", [b_sz, lsh, h_sz], BF16, kind="ExternalOutput").ap()

    with tile.TileContext(nc) as tc, ExitStack() as ctx:
        const = ctx.enter_context(tc.tile_pool(name="const", bufs=1))
        res = ctx.enter_context(tc.tile_pool(name="res", bufs=1))
        work = ctx.enter_context(tc.tile_pool(name="work", bufs=3))
        psum = ctx.enter_context(tc.tile_pool(name="psum", bufs=2, space="PSUM"))

        # Resident SBUF tensors (narrow dtypes straight from DMA, no casts).
        # Masking is exact and multiplicative: host pre-masks x rows (so
        # masked s contribute 0 to mm2) and the rowsum tiny-matmuls use the
        # mask column instead of ones (so masked s leave the denominator).
        # exp() therefore needs no per-chunk bias and fuses into a single
        # ACTIVATE per group.
        XT = res.tile([P, b_sz, KH, s_sz], mm1_dt)  # XT[h%128, b, h//128, s]
        XB = res.tile([P, b_sz, KS, h_sz], BF16)    # XB[s%128, b, s//128, h] (masked)
        WT = res.tile([P, KH, lsh], mm1_dt)         # WT[h%128, h//128, l]
        MC = res.tile([P, b_sz, KS], BF16)          # mask column per s

        def mask_setup():
            nc.sync.dma_start(MC[:], m_d[:])

        # Big input loads go on the ACT HWDGE queue so they stream in
        # parallel with the W-shard loads on the sync queue.
        def xt_setup(b):
            nc.scalar.dma_start(XT[:, b], xt_d[b])

        def xb_setup(b):
            nc.scalar.dma_start(XB[:, b], x_d[b])

        def w_setup(g):
            nc.sync.dma_start(WT[:, :, ts(g, LG)], wt_d[g])

        exp_scale = (1.0 / W_SCALE) if mm1_fp8 else 1.0

        def front(b, g, chunked_exp=False):
            """mm1 (scoresT chunks) + exp for group (b, g).

            exp runs as one fused ACTIVATE over all 4 chunks (saves the
            3x352-cycle ACT ramp); the last group uses per-chunk exps so
            the tail's mm2 isn't serialized behind a 2us fused exp."""
            ps_sct = psum.tile([P, KS, LG], F32, tag="ps_sct", bufs=1)
            exp_g = work.tile([P, KS, LG], BF16, tag="exp", bufs=2)
            for st in range(KS):
                if mm1_fp8:
                    for k2 in range(0, KH, 2):
                        nc.tensor.matmul(
                            ps_sct[:, st, :],
                            XT[:, b, ds(k2, 2), ts(st, P)],
                            WT[:, ds(k2, 2), ts(g, LG)],
                            start=(k2 == 0),
                            stop=(k2 == KH - 2),
                            perf_mode=mybir.MatmulPerfMode.DoubleRow,
                        )
                else:
                    for k in range(KH):
                        nc.tensor.matmul(
                            ps_sct[:, st, :],
                            XT[:, b, k, ts(st, P)],
                            WT[:, k, ts(g, LG)],
                            start=(k == 0),
                            stop=(k == KH - 1),
                        )
                if chunked_exp:
                    nc.scalar.activation(
                        exp_g[:, st, :], ps_sct[:, st, :],
                        mybir.ActivationFunctionType.Exp,
                        scale=exp_scale,
                    )
            if not chunked_exp:
                nc.scalar.activation(
                    exp_g[:], ps_sct[:],
                    mybir.ActivationFunctionType.Exp,
                    scale=exp_scale,
                )
            return exp_g

        def back(b, g, exp_g):
            """mm2 + rowsums + normalize + store for group (b, g)."""
            ps_sums = psum.tile([P, NSUB], F32, tag="ps_sums", bufs=2)
            recips = work.tile([P, NSUB], F32, tag="recips", bufs=4)
            for lt in range(NSUB):
                ps_out = psum.tile([P, h_sz], F32, tag="ps_out", bufs=2)
                for sc in range(KS):
                    stat = exp_g[:, sc, ts(lt, P)]
                    nc.tensor.matmul(
                        ps_out[:], stat, XB[:, b, sc, :],
                        start=(sc == 0), stop=(sc == KS - 1),
                    )
                    nc.tensor.matmul(
                        ps_sums[:, lt : lt + 1], stat, MC[:, b, sc : sc + 1],
                        start=(sc == 0), stop=(sc == KS - 1),
                    )
                nc.vector.reciprocal(
                    recips[:, lt : lt + 1], ps_sums[:, lt : lt + 1]
                )
                out_t = work.tile([P, h_sz], BF16, tag="out", bufs=3)
                nc.scalar.activation(
                    out_t[:], ps_out[:],
                    mybir.ActivationFunctionType.Copy,
                    scale=recips[:, lt : lt + 1],
                )
                nc.sync.dma_start(o_d[b, ds(g * LG + lt * P, P), :], out_t[:])

        # ---- emission. Fill: xt0 streams on the ACT queue while mask+w0+w1
        # go on sync; xb0 follows xt0 on the ACT queue (first read is one
        # step later, in back(0,0)). W groups stream two ahead during b=0;
        # b+1 inputs prefetch early in batch b's pass.
        mask_setup()
        w_setup(0)
        xt_setup(0)
        xb_setup(0)
        w_setup(1)

        pend = [None]
        for b in range(b_sz):
            for g in range(NG):
                if b == 0 and g + 2 < NG:
                    w_setup(g + 2)
                last = (b == b_sz - 1) and (g == NG - 1)
                exp_g = front(b, g, chunked_exp=last)
                if b < b_sz - 1 and g == 0:
                    xt_setup(b + 1)
                    xb_setup(b + 1)
                if pend[0] is not None:
                    back(*pend[0])
                pend[0] = (b, g, exp_g)
        back(*pend[0])

    nc.compile()
    return nc


_CACHE = {}

VARIANT = "d"  # "c": bf16 mm1; "d": fp8 DoubleRow mm1


def _get_module():
    if VARIANT not in _CACHE:
        _CACHE[VARIANT] = build_module(mm1_fp8=(VARIANT == "d"))
    return _CACHE[VARIANT]


def _run(inputs: np.ndarray, masks: np.ndarray, W: np.ndarray, **spmd_kwargs):
    """Run on 8 cores; returns (full output, BassKernelResults)."""
    nc = _get_module()

    P, KS, KH, LG = 128, S // 128, H // 128, 512
    NG = LSH // LG
    x32 = np.ascontiguousarray(inputs, dtype=np.float32)
    mf = np.ascontiguousarray(masks, dtype=np.float32)
    # Pre-mask x rows: masked s contribute 0 to mm2 and (via the mask
    # column in the rowsum matmuls) to the softmax denominator -- exact
    # equivalent of -inf score masking.
    xm = x32 * mf[:, :, None]
    # Pack into per-partition-contiguous SBUF layouts (see build_module).
    x = np.ascontiguousarray(
        xm.reshape(B, KS, P, H).swapaxes(1, 2)
    ).astype(ml_dtypes.bfloat16)                       # [B, P, KS, H]
    xt = np.ascontiguousarray(
        np.swapaxes(x32, 1, 2).reshape(B, KH, P, S).swapaxes(1, 2)
    )                                                  # [B, P, KH, S] f32
    mc = np.ascontiguousarray(
        mf.reshape(B, KS, P).transpose(2, 0, 1)
    ).astype(ml_dtypes.bfloat16)                       # [P, B, KS]
    wt_pad = np.zeros((H, L_PAD), dtype=np.float32)
    wt_pad[:, :L] = W.T
    if VARIANT == "d":
        xt = xt.astype(ml_dtypes.float8_e4m3)
        wt_pad = np.clip(wt_pad * W_SCALE, -240.0, 240.0)
        wdt = ml_dtypes.float8_e4m3
    else:
        xt = xt.astype(ml_dtypes.bfloat16)
        wdt = ml_dtypes.bfloat16

    def pack_w(c):
        # [H, LSH] shard -> [NG, P, KH, LG]
        shard = wt_pad[:, c * LSH : (c + 1) * LSH]
        return np.ascontiguousarray(
            shard.reshape(KH, P, NG, LG).transpose(2, 1, 0, 3)
        ).astype(wdt)

    in_maps = [
        {"x": x, "xt": xt, "m": mc, "wt": pack_w(c)}
        for c in range(N_CORES)
    ]
    res = bass_utils.run_bass_kernel_spmd(
        nc, in_maps, core_ids=list(range(N_CORES)), **spmd_kwargs
    )
    out = np.concatenate(
        [res.results[c]["o"].astype(np.float32) for c in range(N_CORES)], axis=1
    )
    return np.ascontiguousarray(out[:, :L, :]), res


def kernel(inputs: np.ndarray, masks: np.ndarray, W: np.ndarray) -> np.ndarray:
    out, _ = _run(inputs, masks, W)
    return out


# revision 24
# speedup vs baseline: 1.3325x; 1.0955x over previous
"""MLAttention (label-pooling attention) Trainium2 Bass kernel.

Computes, for full inputs:
    scores = einsum('bsh,lh->bls', inputs, W)
    scores = where(mask==0, -inf, scores)
    attn   = softmax(scores, axis=-1)
    out    = einsum('bls,bsh->blh', attn, inputs)

Label-parallel across 8 NeuronCores: L=28415 padded to 28672 = 8*3584.
Each core gets its own W shard [3584, 512]; inputs/masks replicated.
Host concatenates the 8 per-core outputs [B, 3584, H] and trims to L.

Transpose-free dataflow. Scores are computed TRANSPOSED, in [s, l]
layout, so the exp() tile is directly the stationary operand of the
second matmul -- no PE transposes. The softmax mask is folded into the
exp bias (per-partition = per-s). Row-sums (softmax denominators) come
from N=1 matmuls against a ones column that reuse mm2's already-loaded
stationary, accumulating into a separate PSUM bank; normalization
happens in the final ACT copy via a per-partition reciprocal scale.

Matmul operands are host-cast: mm2 in bf16; mm1 either bf16 ("c") or
fp8e4m3 with DoubleRow perf mode ("d", 2 MACs/cell/cycle, halves mm1
stream time; W is pre-scaled by 2^14 into fp8 range and the exp
activation descales via its free affine scale). Accumulation is fp32
in PSUM; exp() runs on ACT in fp32 from PSUM and rounds to bf16.

Input DMAs are issued on the ACT HWDGE queue, W-shard and output DMAs
on the sync queue, so the two big fill-phase loads stream in parallel.
A one-step software pipeline (group g's mm1 emitted before group
g-1's mm2) keeps the in-order PE queue full while g's exp chain
completes on ACT.
"""

from contextlib import ExitStack

import ml_dtypes
import numpy as np

import concourse.bass as bass
import concourse.mybir as mybir
import concourse.tile as tile
from concourse import bacc, bass_utils
from concourse.bass import ds, ts

F32 = mybir.dt.float32
BF16 = mybir.dt.bfloat16
FP8 = mybir.dt.float8e4

# Problem shapes (hardcoded per contract).
B, S, H, L = 4, 512, 512, 28415
N_CORES = 8
LSH = 3584               # per-core padded label count (28 tiles of 128)
L_PAD = LSH * N_CORES    # 28672
W_SCALE = 2.0 ** 14      # fp8 variant: host premultiplies W, exp descales


def build_module(b_sz=B, s_sz=S, h_sz=H, lsh=LSH, n_devices=N_CORES,
                 mm1_fp8=False):
    P = 128
    KH = h_sz // P   # H contraction chunks (mm1)
    KS = s_sz // P   # S contraction chunks (mm2) == score s-tiles
    LG = 512         # label group per step
    NG = lsh // LG   # groups per batch
    NSUB = LG // P   # 128-label tiles per group
    mm1_dt = FP8 if mm1_fp8 else BF16

    nc = bacc.Bacc(
        "TRN2", target_bir_lowering=False, debug=False, num_devices=n_devices
    )
    # Inputs are host-packed into the exact SBUF per-partition layouts so
    # every DMA moves 2-4 KiB contiguous runs per partition (full HBM BW).
    x_d = nc.dram_tensor(
        "x", [b_sz, P, KS, h_sz], BF16, kind="ExternalInput"
    ).ap()
    xt_d = nc.dram_tensor(
        "xt", [b_sz, P, KH, s_sz], mm1_dt, kind="ExternalInput"
    ).ap()
    wt_d = nc.dram_tensor(
        "wt", [NG, P, KH, LG], mm1_dt, kind="ExternalInput"
    ).ap()
    m_d = nc.dram_tensor("m", [P, b_sz, KS], BF16, kind="ExternalInput").ap()
    o_d = nc.dram_tensor("o", [b_sz, lsh, h_sz], BF16, kind="ExternalOutput").ap()

    with tile.TileContext(nc) as tc, ExitStack() as ctx:
        const = ctx.enter_context(tc.tile_pool(name="const", bufs=1))
        res = ctx.enter_context(tc.tile_pool(name="res", bufs=1))
        work = ctx.enter_context(tc.tile_pool(name="work", bufs=3))
        psum = ctx.enter_context(tc.tile_pool(name="psum", bufs=2, space="PSUM"))

        # Resident SBUF tensors (narrow dtypes straight from DMA, no casts).
        # Masking is exact and multiplicative: host pre-masks x rows (so
        # masked s contribute 0 to mm2) and the rowsum tiny-matmuls use the
        # mask column instead of ones (so masked s leave the denominator).
        # exp() therefore needs no per-chunk bias and fuses into a single
        # ACTIVATE per group.
        XT = res.tile([P, b_sz, KH, s_sz], mm1_dt)  # XT[h%128, b, h//128, s]
        XB = res.tile([P, b_sz, KS, h_sz], BF16)    # XB[s%128, b, s//128, h] (masked)
        WT = res.tile([P, KH, lsh], mm1_dt)         # WT[h%128, h//128, l]
        MC = res.tile([P, b_sz, KS], BF16)          # mask column per s

        def mask_setup():
            nc.sync.dma_start(MC[:], m_d[:])

        # Big input loads go on the ACT HWDGE queue so they stream in
        # parallel with the W-shard loads on the sync queue.
        def xt_setup(b):
            nc.scalar.dma_start(XT[:, b], xt_d[b])

        def xb_setup(b):
            nc.scalar.dma_start(XB[:, b], x_d[b])

        def w_setup(g):
            nc.sync.dma_start(WT[:, :, ts(g, LG)], wt_d[g])

        exp_scale = (1.0 / W_SCALE) if mm1_fp8 else 1.0

        def front(b, g):
            """mm1 (scoresT chunks) + per-chunk exp for group (b, g).

            Per-chunk exps release each ps_sct bank as soon as it's
            consumed, so group g+1's mm1 never stalls on the WAR."""
            ps_sct = psum.tile([P, KS, LG], F32, tag="ps_sct", bufs=1)
            exp_g = work.tile([P, KS, LG], BF16, tag="exp", bufs=2)
            for st in range(KS):
                if mm1_fp8:
                    for k2 in range(0, KH, 2):
                        nc.tensor.matmul(
                            ps_sct[:, st, :],
                            XT[:, b, ds(k2, 2), ts(st, P)],
                            WT[:, ds(k2, 2), ts(g, LG)],
                            start=(k2 == 0),
                            stop=(k2 == KH - 2),
                            perf_mode=mybir.MatmulPerfMode.DoubleRow,
                        )
                else:
                    for k in range(KH):
                        nc.tensor.matmul(
                            ps_sct[:, st, :],
                            XT[:, b, k, ts(st, P)],
                            WT[:, k, ts(g, LG)],
                            start=(k == 0),
                            stop=(k == KH - 1),
                        )
                nc.scalar.activation(
                    exp_g[:, st, :], ps_sct[:, st, :],
                    mybir.ActivationFunctionType.Exp,
                    scale=exp_scale,
                )
            return exp_g

        def back(b, g, exp_g):
            """mm2 + rowsums + normalize + store for group (b, g).

            Rowsums run first (all 16 tiny MMs, then ONE reciprocal), so
            the recips are long done when the copies need them and the
            ps_sums bank WAR never binds. ps_sums is padded to a full
            PSUM bank so its two buffers never share a bank (PE-write vs
            DVE-read serialization). The 4 normalize copies alternate
            ACT/DVE to keep ACT off the critical path."""
            ps_sums = psum.tile([P, 512], F32, tag="ps_sums", bufs=2)
            recips = work.tile([P, NSUB], F32, tag="recips", bufs=2)
            for lt in range(NSUB):
                for sc in range(KS):
                    nc.tensor.matmul(
                        ps_sums[:, lt : lt + 1],
                        exp_g[:, sc, ts(lt, P)],
                        MC[:, b, sc : sc + 1],
                        start=(sc == 0), stop=(sc == KS - 1),
                    )
            nc.vector.reciprocal(recips[:], ps_sums[:, 0:NSUB])
            for lt in range(NSUB):
                ps_out = psum.tile([P, h_sz], F32, tag="ps_out", bufs=2)
                for sc in range(KS):
                    nc.tensor.matmul(
                        ps_out[:], exp_g[:, sc, ts(lt, P)], XB[:, b, sc, :],
                        start=(sc == 0), stop=(sc == KS - 1),
                    )
                out_t = work.tile([P, h_sz], BF16, tag="out", bufs=3)
                if lt % 2 == 0:
                    nc.scalar.activation(
                        out_t[:], ps_out[:],
                        mybir.ActivationFunctionType.Copy,
                        scale=recips[:, lt : lt + 1],
                    )
                else:
                    nc.vector.tensor_scalar_mul(
                        out=out_t[:], in0=ps_out[:],
                        scalar1=recips[:, lt : lt + 1],
                    )
                nc.sync.dma_start(o_d[b, ds(g * LG + lt * P, P), :], out_t[:])

        # ---- emission. Fill: xt0 streams on the ACT queue while mask+w0+w1
        # go on sync; xb0 follows xt0 on the ACT queue (first read is one
        # step later, in back(0,0)). W groups stream two ahead during b=0;
        # b+1 inputs prefetch early in batch b's pass.
        mask_setup()
        w_setup(0)
        xt_setup(0)
        xb_setup(0)
        w_setup(1)

        pend = [None]
        for b in range(b_sz):
            for g in range(NG):
                if b == 0 and g + 2 < NG:
                    w_setup(g + 2)
                exp_g = front(b, g)
                if b < b_sz - 1 and g == 0:
                    xt_setup(b + 1)
                    xb_setup(b + 1)
                if pend[0] is not None:
                    back(*pend[0])
                pend[0] = (b, g, exp_g)
        back(*pend[0])

    nc.compile()
    return nc


_CACHE = {}

VARIANT = "d"  # "c": bf16 mm1; "d": fp8 DoubleRow mm1


def _get_module():
    if VARIANT not in _CACHE:
        _CACHE[VARIANT] = build_module(mm1_fp8=(VARIANT == "d"))
    return _CACHE[VARIANT]


def _run(inputs: np.ndarray, masks: np.ndarray, W: np.ndarray, **spmd_kwargs):
    """Run on 8 cores; returns (full output, BassKernelResults)."""
    nc = _get_module()

    P, KS, KH, LG = 128, S // 128, H // 128, 512
    NG = LSH // LG
    x32 = np.ascontiguousarray(inputs, dtype=np.float32)
    mf = np.ascontiguousarray(masks, dtype=np.float32)
    # Pre-mask x rows: masked s contribute 0 to mm2 and (via the mask
    # column in the rowsum matmuls) to the softmax denominator -- exact
    # equivalent of -inf score masking.
    xm = x32 * mf[:, :, None]
    # Pack into per-partition-contiguous SBUF layouts (see build_module).
    x = np.ascontiguousarray(
        xm.reshape(B, KS, P, H).swapaxes(1, 2)
    ).astype(ml_dtypes.bfloat16)                       # [B, P, KS, H]
    xt = np.ascontiguousarray(
        np.swapaxes(x32, 1, 2).reshape(B, KH, P, S).swapaxes(1, 2)
    )                                                  # [B, P, KH, S] f32
    mc = np.ascontiguousarray(
        mf.reshape(B, KS, P).transpose(2, 0, 1)
    ).astype(ml_dtypes.bfloat16)                       # [P, B, KS]
    wt_pad = np.zeros((H, L_PAD), dtype=np.float32)
    wt_pad[:, :L] = W.T
    if VARIANT == "d":
        xt = xt.astype(ml_dtypes.float8_e4m3)
        wt_pad = np.clip(wt_pad * W_SCALE, -240.0, 240.0)
        wdt = ml_dtypes.float8_e4m3
    else:
        xt = xt.astype(ml_dtypes.bfloat16)
        wdt = ml_dtypes.bfloat16

    def pack_w(c):
        # [H, LSH] shard -> [NG, P, KH, LG]
        shard = wt_pad[:, c * LSH : (c + 1) * LSH]
        return np.ascontiguousarray(
            shard.reshape(KH, P, NG, LG).transpose(2, 1, 0, 3)
        ).astype(wdt)

    in_maps = [
        {"x": x, "xt": xt, "m": mc, "wt": pack_w(c)}
        for c in range(N_CORES)
    ]
    res = bass_utils.run_bass_kernel_spmd(
        nc, in_maps, core_ids=list(range(N_CORES)), **spmd_kwargs
    )
    out = np.concatenate(
        [res.results[c]["o"].astype(np.float32) for c in range(N_CORES)], axis=1
    )
    return np.ascontiguousarray(out[:, :L, :]), res


def kernel(inputs: np.ndarray, masks: np.ndarray, W: np.ndarray) -> np.ndarray:
    out, _ = _run(inputs, masks, W)
    return out


# revision 29
# speedup vs baseline: 1.3514x; 1.0142x over previous
"""MLAttention (label-pooling attention) Trainium2 Bass kernel.

Computes, for full inputs:
    scores = einsum('bsh,lh->bls', inputs, W)
    scores = where(mask==0, -inf, scores)
    attn   = softmax(scores, axis=-1)
    out    = einsum('bls,bsh->blh', attn, inputs)

Label-parallel across 8 NeuronCores: L=28415 padded to 28672 = 8*3584.
Each core gets its own W shard [3584, 512]; inputs/masks replicated.
Host concatenates the 8 per-core outputs [B, 3584, H] and trims to L.

Transpose-free dataflow. Scores are computed TRANSPOSED, in [s, l]
layout, so the exp() tile is directly the stationary operand of the
second matmul -- no PE transposes. The softmax mask is folded into the
exp bias (per-partition = per-s). Row-sums (softmax denominators) come
from N=1 matmuls against a ones column that reuse mm2's already-loaded
stationary, accumulating into a separate PSUM bank; normalization
happens in the final ACT copy via a per-partition reciprocal scale.

Matmul operands are host-cast: mm2 in bf16; mm1 either bf16 ("c") or
fp8e4m3 with DoubleRow perf mode ("d", 2 MACs/cell/cycle, halves mm1
stream time; W is pre-scaled by 2^14 into fp8 range and the exp
activation descales via its free affine scale). Accumulation is fp32
in PSUM; exp() runs on ACT in fp32 from PSUM and rounds to bf16.

Input DMAs are issued on the ACT HWDGE queue, W-shard and output DMAs
on the sync queue, so the two big fill-phase loads stream in parallel.
A one-step software pipeline (group g's mm1 emitted before group
g-1's mm2) keeps the in-order PE queue full while g's exp chain
completes on ACT.
"""

from contextlib import ExitStack

import ml_dtypes
import numpy as np

import concourse.bass as bass
import concourse.mybir as mybir
import concourse.tile as tile
from concourse import bacc, bass_utils
from concourse.bass import ds, ts

F32 = mybir.dt.float32
BF16 = mybir.dt.bfloat16
FP8 = mybir.dt.float8e4

# Problem shapes (hardcoded per contract).
B, S, H, L = 4, 512, 512, 28415
N_CORES = 8
LSH = 3584               # per-core padded label count (28 tiles of 128)
L_PAD = LSH * N_CORES    # 28672
W_SCALE = 2.0 ** 14      # fp8 variant: host premultiplies W, exp descales


def build_module(b_sz=B, s_sz=S, h_sz=H, lsh=LSH, n_devices=N_CORES,
                 mm1_fp8=False):
    P = 128
    KH = h_sz // P   # H contraction chunks (mm1)
    KS = s_sz // P   # S contraction chunks (mm2) == score s-tiles
    LG = 512         # label group per step
    NG = lsh // LG   # groups per batch
    NSUB = LG // P   # 128-label tiles per group
    mm1_dt = FP8 if mm1_fp8 else BF16

    nc = bacc.Bacc(
        "TRN2", target_bir_lowering=False, debug=False, num_devices=n_devices
    )
    # Inputs are host-packed into the exact SBUF per-partition layouts so
    # every DMA moves 2-4 KiB contiguous runs per partition (full HBM BW).
    x_d = nc.dram_tensor(
        "x", [b_sz, P, KS, h_sz], BF16, kind="ExternalInput"
    ).ap()
    xt_d = nc.dram_tensor(
        "xt", [b_sz, P, KH, s_sz], mm1_dt, kind="ExternalInput"
    ).ap()
    wt_d = nc.dram_tensor(
        "wt", [NG, P, KH, LG], mm1_dt, kind="ExternalInput"
    ).ap()
    m_d = nc.dram_tensor("m", [P, b_sz, KS], BF16, kind="ExternalInput").ap()
    o_d = nc.dram_tensor("o", [b_sz, lsh, h_sz], BF16, kind="ExternalOutput").ap()

    with tile.TileContext(nc) as tc, ExitStack() as ctx:
        const = ctx.enter_context(tc.tile_pool(name="const", bufs=1))
        res = ctx.enter_context(tc.tile_pool(name="res", bufs=1))
        work = ctx.enter_context(tc.tile_pool(name="work", bufs=3))
        psum = ctx.enter_context(tc.tile_pool(name="psum", bufs=2, space="PSUM"))

        # Resident SBUF tensors (narrow dtypes straight from DMA, no casts).
        # Masking is exact and multiplicative: host pre-masks x rows (so
        # masked s contribute 0 to mm2) and the rowsum tiny-matmuls use the
        # mask column instead of ones (so masked s leave the denominator).
        # exp() therefore needs no per-chunk bias and fuses into a single
        # ACTIVATE per group.
        XT = res.tile([P, b_sz, KH, s_sz], mm1_dt)  # XT[h%128, b, h//128, s]
        XB = res.tile([P, b_sz, KS, h_sz], BF16)    # XB[s%128, b, s//128, h] (masked)
        WT = res.tile([P, KH, lsh], mm1_dt)         # WT[h%128, h//128, l]
        MC = res.tile([P, b_sz, KS], BF16)          # mask column per s

        def mask_setup():
            nc.sync.dma_start(MC[:], m_d[:])

        # Big input loads go on the ACT HWDGE queue so they stream in
        # parallel with the W-shard loads on the sync queue.
        def xt_setup(b):
            nc.scalar.dma_start(XT[:, b], xt_d[b])

        def xb_setup(b):
            nc.scalar.dma_start(XB[:, b], x_d[b])

        def w_setup(g):
            nc.sync.dma_start(WT[:, :, ts(g, LG)], wt_d[g])

        exp_scale = (1.0 / W_SCALE) if mm1_fp8 else 1.0

        def front(b, g):
            """mm1 (scoresT chunks) + per-chunk exp for group (b, g).

            Per-chunk exps release each ps_sct bank as soon as it's
            consumed, so group g+1's mm1 never stalls on the WAR."""
            ps_sct = psum.tile([P, KS, LG], F32, tag="ps_sct", bufs=1)
            exp_g = work.tile([P, KS, LG], BF16, tag="exp", bufs=2)
            for st in range(KS):
                if mm1_fp8:
                    for k2 in range(0, KH, 2):
                        nc.tensor.matmul(
                            ps_sct[:, st, :],
                            XT[:, b, ds(k2, 2), ts(st, P)],
                            WT[:, ds(k2, 2), ts(g, LG)],
                            start=(k2 == 0),
                            stop=(k2 == KH - 2),
                            perf_mode=mybir.MatmulPerfMode.DoubleRow,
                        )
                else:
                    for k in range(KH):
                        nc.tensor.matmul(
                            ps_sct[:, st, :],
                            XT[:, b, k, ts(st, P)],
                            WT[:, k, ts(g, LG)],
                            start=(k == 0),
                            stop=(k == KH - 1),
                        )
                nc.scalar.activation(
                    exp_g[:, st, :], ps_sct[:, st, :],
                    mybir.ActivationFunctionType.Exp,
                    scale=exp_scale,
                )
            return exp_g

        def back(b, g, exp_g, split_dma=False):
            """mm2 + rowsums + normalize + store for group (b, g).

            Rowsums run first (all 16 tiny MMs, then ONE reciprocal), so
            the recips are long done when the copies need them and the
            ps_sums bank WAR never binds. ps_sums is padded to a full
            PSUM bank so its two buffers never share a bank (PE-write vs
            DVE-read serialization). The 4 normalize copies alternate
            ACT/DVE to keep ACT off the critical path."""
            ps_sums = psum.tile([P, 512], F32, tag="ps_sums", bufs=2)
            recips = work.tile([P, NSUB], F32, tag="recips", bufs=2)
            for lt in range(NSUB):
                for sc in range(KS):
                    nc.tensor.matmul(
                        ps_sums[:, lt : lt + 1],
                        exp_g[:, sc, ts(lt, P)],
                        MC[:, b, sc : sc + 1],
                        start=(sc == 0), stop=(sc == KS - 1),
                    )
            nc.vector.reciprocal(recips[:], ps_sums[:, 0:NSUB])
            for lt in range(NSUB):
                ps_out = psum.tile([P, h_sz], F32, tag="ps_out", bufs=2)
                for sc in range(KS):
                    nc.tensor.matmul(
                        ps_out[:], exp_g[:, sc, ts(lt, P)], XB[:, b, sc, :],
                        start=(sc == 0), stop=(sc == KS - 1),
                    )
                out_t = work.tile([P, h_sz], BF16, tag="out", bufs=3)
                if lt % 2 == 0:
                    nc.scalar.activation(
                        out_t[:], ps_out[:],
                        mybir.ActivationFunctionType.Copy,
                        scale=recips[:, lt : lt + 1],
                    )
                else:
                    nc.vector.tensor_scalar_mul(
                        out=out_t[:], in0=ps_out[:],
                        scalar1=recips[:, lt : lt + 1],
                    )
                # For the final group, split output DMAs across both HWDGE
                # queues so the tail flush halves. (Not done in steady
                # state: a DMA issue costs ~700ns of ACT engine time.)
                q = nc.scalar if (split_dma and lt % 2) else nc.sync
                q.dma_start(o_d[b, ds(g * LG + lt * P, P), :], out_t[:])

        # ---- emission. Fill: xt0 streams on the ACT queue while w0+w1 go
        # on sync; xb0 follows xt0 on the ACT queue (first read is one
        # step later, in back(0,0)); the mask load is small and only
        # needed by back(0,0), so it queues after the critical W loads.
        # W groups stream two ahead during b=0; b+1 inputs prefetch early
        # in batch b's pass.
        w_setup(0)
        xt_setup(0)
        xb_setup(0)
        w_setup(1)
        mask_setup()

        pend = [None]
        for b in range(b_sz):
            for g in range(NG):
                if b == 0 and g + 2 < NG:
                    w_setup(g + 2)
                exp_g = front(b, g)
                if b < b_sz - 1 and g == 0:
                    xt_setup(b + 1)
                    xb_setup(b + 1)
                if pend[0] is not None:
                    pb, pg, pe_ = pend[0]
                    back(pb, pg, pe_, split_dma=(pb == b_sz - 1 and pg >= NG - 2))
                pend[0] = (b, g, exp_g)
        pb, pg, pe_ = pend[0]
        back(pb, pg, pe_, split_dma=True)

    nc.compile()
    return nc


_CACHE = {}

VARIANT = "d"  # "c": bf16 mm1; "d": fp8 DoubleRow mm1


def _get_module():
    if VARIANT not in _CACHE:
        _CACHE[VARIANT] = build_module(mm1_fp8=(VARIANT == "d"))
    return _CACHE[VARIANT]


def _run(inputs: np.ndarray, masks: np.ndarray, W: np.ndarray, **spmd_kwargs):
    """Run on 8 cores; returns (full output, BassKernelResults)."""
    nc = _get_module()

    P, KS, KH, LG = 128, S // 128, H // 128, 512
    NG = LSH // LG
    x32 = np.ascontiguousarray(inputs, dtype=np.float32)
    mf = np.ascontiguousarray(masks, dtype=np.float32)
    # Pre-mask x rows: masked s contribute 0 to mm2 and (via the mask
    # column in the rowsum matmuls) to the softmax denominator -- exact
    # equivalent of -inf score masking.
    xm = x32 * mf[:, :, None]
    # Pack into per-partition-contiguous SBUF layouts (see build_module).
    x = np.ascontiguousarray(
        xm.reshape(B, KS, P, H).swapaxes(1, 2)
    ).astype(ml_dtypes.bfloat16)                       # [B, P, KS, H]
    xt = np.ascontiguousarray(
        np.swapaxes(x32, 1, 2).reshape(B, KH, P, S).swapaxes(1, 2)
    )                                                  # [B, P, KH, S] f32
    mc = np.ascontiguousarray(
        mf.reshape(B, KS, P).transpose(2, 0, 1)
    ).astype(ml_dtypes.bfloat16)                       # [P, B, KS]
    wt_pad = np.zeros((H, L_PAD), dtype=np.float32)
    wt_pad[:, :L] = W.T
    if VARIANT == "d":
        xt = xt.astype(ml_dtypes.float8_e4m3)
        wt_pad = np.clip(wt_pad * W_SCALE, -240.0, 240.0)
        wdt = ml_dtypes.float8_e4m3
    else:
        xt = xt.astype(ml_dtypes.bfloat16)
        wdt = ml_dtypes.bfloat16

    def pack_w(c):
        # [H, LSH] shard -> [NG, P, KH, LG]
        shard = wt_pad[:, c * LSH : (c + 1) * LSH]
        return np.ascontiguousarray(
            shard.reshape(KH, P, NG, LG).transpose(2, 1, 0, 3)
        ).astype(wdt)

    in_maps = [
        {"x": x, "xt": xt, "m": mc, "wt": pack_w(c)}
        for c in range(N_CORES)
    ]
    res = bass_utils.run_bass_kernel_spmd(
        nc, in_maps, core_ids=list(range(N_CORES)), **spmd_kwargs
    )
    out = np.concatenate(
        [res.results[c]["o"].astype(np.float32) for c in range(N_CORES)], axis=1
    )
    return np.ascontiguousarray(out[:, :L, :]), res


def kernel(inputs: np.ndarray, masks: np.ndarray, W: np.ndarray) -> np.ndarray:
    out, _ = _run(inputs, masks, W)
    return out
